# revision 13
# baseline (speedup 1.0000x reference)
"""Trainium2 Bass kernel for nn_DecoderLSTM (2-layer LSTM decoder, B=256, F=64,
H=1024, T=96) on 8 NeuronCores.

Strategy: tensor-parallel over the 4H gate dimension (the three big weight
matrices total 50MB fp32 and do not fit in one core's 24MB SBUF). Each core
holds a 512-row slice of every gate matrix (rows [g*H + 128r, g*H + 128r + 128)
for gate g in i,f,g,o), computes its gate slice for the FULL batch B=256
(full 128-wide PE utilization), updates its h/c slice, transposes it on the PE,
and AllGathers the transposed hidden state (h^T, [1024, 256]) once per layer
per step so every core has the full contraction operand for the next matmuls.

The per-step output projection x_t = pred_{t-1} = h1 @ W_fc.T + b_fc is folded
into layer 0's weights (W0eff = W_ih0_slice @ W_fc), removing the FC from the
recurrent critical path; pred_t itself is computed per-step as an 8-row slice
per core (predT = W_fc[8r:8r+8] @ h1^T) so the 8 cores jointly produce the
full [B, T, F] output with no extra gather.

Matmuls run in float32r (TF32-like fast fp32 mode, 4x the fp32 rate, ~1e-4
relative error); the cell state c stays fp32.
"""
import sys

sys.path.insert(0, "/opt/trn_rl_repo")

import numpy as np
import concourse.bass as bass
import concourse.bacc as bacc
import concourse.mybir as mybir
import concourse.tile as tile
from concourse.bass_utils import run_bass_kernel_spmd

dt = mybir.dt

N_CORES = 8
B = 256
F = 64
H = 1024
NG = 512          # per-core gate rows (4 gates x 128)
SL = 128          # per-core within-gate slice (H / N_CORES)
KT = H // 128     # 8 K-tiles per state
FS = F // N_CORES  # 8 pred rows per core

_BUILD_CACHE = {}


def _build(T: int):
    """Build + compile the SPMD program for T timesteps."""
    nc = bacc.Bacc(None, num_devices=N_CORES)
    f32, f32r = dt.float32, dt.float32r

    # ---- per-core external inputs ----
    w0_in = nc.dram_tensor("w0_in", [16, 128, NG], f32, kind="ExternalInput")
    w1_in = nc.dram_tensor("w1_in", [16, 128, NG], f32, kind="ExternalInput")
    wih0_in = nc.dram_tensor("wih0_in", [F, NG], f32, kind="ExternalInput")
    brows_in = nc.dram_tensor("brows_in", [3, NG], f32, kind="ExternalInput")  # b0_t0, b0eff, b1
    wfc_in = nc.dram_tensor("wfc_in", [H, FS], f32, kind="ExternalInput")
    bfc_in = nc.dram_tensor("bfc_in", [FS, 1], f32, kind="ExternalInput")
    x0t_in = nc.dram_tensor("x0t_in", [F, B], f32, kind="ExternalInput")
    h0t_in = nc.dram_tensor("h0t_in", [H, B], f32, kind="ExternalInput")
    h1t_in = nc.dram_tensor("h1t_in", [H, B], f32, kind="ExternalInput")
    c0_in = nc.dram_tensor("c0_in", [B, SL], f32, kind="ExternalInput")
    c1_in = nc.dram_tensor("c1_in", [B, SL], f32, kind="ExternalInput")
    ones_in = nc.dram_tensor("ones_in", [1, 128], f32, kind="ExternalInput")
    eye_in = nc.dram_tensor("eye_in", [128, 128], f32, kind="ExternalInput")

    preds_out = nc.dram_tensor("preds", [T, FS, B], f32, kind="ExternalOutput")

    with tile.TileContext(nc) as tc:
        with (
            tc.tile_pool(name="wpool", bufs=1) as wpool,      # persistent weights
            tc.tile_pool(name="state", bufs=2) as state,      # hT double buffers
            tc.tile_pool(name="cst", bufs=2) as cst,          # c state tiles
            tc.tile_pool(name="act", bufs=3) as actp,         # activation tiles
            tc.tile_pool(name="tmp", bufs=3) as tmp,          # small temporaries
            tc.tile_pool(name="psg", bufs=1, space="PSUM") as psg,
            tc.tile_pool(name="pst", bufs=2, space="PSUM") as pst,
            tc.tile_pool(name="psf", bufs=2, space="PSUM") as psf,
            tc.tile_pool(name="agd", bufs=4, space="DRAM") as agd,
        ):
            # ================= init: load + round weights to f32r =============
            w0_r = wpool.tile([128, 16 * NG], f32r)
            w1_r = wpool.tile([128, 16 * NG], f32r)
            with tc.tile_pool(name="stage", bufs=2) as stage:
                for src, dst in ((w0_in, w0_r), (w1_in, w1_r)):
                    for k in range(16):
                        st = stage.tile([128, NG], f32, tag="wst")
                        nc.sync.dma_start(st[:], src[k, :, :])
                        nc.vector.tensor_copy(dst[:, k * NG:(k + 1) * NG], st[:])

                wih0_r = wpool.tile([F, NG], f32r)
                st = stage.tile([F, NG], f32, tag="mst")
                nc.sync.dma_start(st[:], wih0_in[:])
                nc.vector.tensor_copy(wih0_r[:], st[:])

                brow_tiles = []
                for bi in range(3):
                    br = wpool.tile([1, NG], f32r, tag=f"brow{bi}", name=f"brow{bi}")
                    st = stage.tile([1, NG], f32, tag="mst")
                    nc.sync.dma_start(st[:], brows_in[bi:bi + 1, :])
                    nc.vector.tensor_copy(br[:], st[:])
                    brow_tiles.append(br)

                wfc_r = wpool.tile([128, KT * FS], f32r)
                st = stage.tile([128, KT * FS], f32, tag="mst")
                nc.sync.dma_start(
                    st[:].rearrange("p (k f) -> p k f", k=KT),
                    wfc_in[:].rearrange("(k p) f -> p k f", p=128),
                )
                nc.vector.tensor_copy(wfc_r[:], st[:])

                bfc_sb = wpool.tile([FS, 1], f32)
                nc.sync.dma_start(bfc_sb[:], bfc_in[:])

                x0t_r = wpool.tile([F, B], f32r)
                st = stage.tile([F, B], f32, tag="mst")
                nc.sync.dma_start(st[:], x0t_in[:])
                nc.vector.tensor_copy(x0t_r[:], st[:])

                ones_r = wpool.tile([1, 128], f32r)
                st = stage.tile([1, 128], f32, tag="mst")
                nc.sync.dma_start(st[:], ones_in[:])
                nc.vector.tensor_copy(ones_r[:], st[:])

                eye_sb = wpool.tile([128, 128], f32)
                nc.sync.dma_start(eye_sb[:], eye_in[:])

                # initial transposed states (full, gathered layout) and c slices
                h0t_cur = state.tile([128, KT * B], f32r, tag="h0t")
                h1t_cur = state.tile([128, KT * B], f32r, tag="h1t")
                for src, dst in ((h0t_in, h0t_cur), (h1t_in, h1t_cur)):
                    st = stage.tile([128, KT * B], f32, tag="hst")
                    nc.sync.dma_start(
                        st[:].rearrange("p (k b) -> p k b", k=KT),
                        src[:].rearrange("(k p) b -> p k b", p=128),
                    )
                    nc.vector.tensor_copy(dst[:], st[:])

                c_cur = [[None, None], [None, None]]
                for li, src in ((0, c0_in), (1, c1_in)):
                    for m in range(2):
                        ct = cst.tile([128, SL], f32, tag=f"c{li}{m}")
                        nc.sync.dma_start(ct[:], src[m * 128:(m + 1) * 128, :])
                        c_cur[li][m] = ct

            # bias rows ([1, NG] f32r)
            brow0_t0 = brow_tiles[0][:]
            brow0 = brow_tiles[1][:]
            brow1 = brow_tiles[2][:]

            # ================= recurrent steps ================================
            for t in range(T):
                # ---- layer 0 gates: [2][128, 512] psum ----
                g0 = [psg.tile([128, NG], f32, tag=f"g0{m}", name=f"g0{m}") for m in range(2)]
                for m in range(2):
                    # h0 part (k-tiles 0..7) — prev-step h0T, gathered early
                    for k in range(KT):
                        nc.tensor.matmul(
                            g0[m][:],
                            h0t_cur[:, k * B + m * 128: k * B + m * 128 + 128],
                            w0_r[:, k * NG:(k + 1) * NG],
                            start=(k == 0), stop=False,
                        )
                    if t == 0:
                        # x0 path: K=64 stationary + its own bias row
                        nc.tensor.matmul(
                            g0[m][:], x0t_r[:, m * 128: m * 128 + 128],
                            wih0_r[:], start=False, stop=False,
                        )
                        nc.tensor.matmul(
                            g0[m][:], ones_r[:], brow0_t0,
                            start=False, stop=True,
                        )
                    else:
                        # h1 part via fused W0eff (k-tiles 8..15) + bias row
                        for k in range(KT):
                            nc.tensor.matmul(
                                g0[m][:],
                                h1t_cur[:, k * B + m * 128: k * B + m * 128 + 128],
                                w0_r[:, (8 + k) * NG:(9 + k) * NG],
                                start=False, stop=False,
                            )
                        nc.tensor.matmul(
                            g0[m][:], ones_r[:], brow0,
                            start=False, stop=True,
                        )

                # ---- layer 0 activations + cell update + h0 ----
                ag_in0 = agd.tile([128, B], f32r, tag="agi0")
                hto0 = tmp.tile([128, B], f32r, tag="hto0")
                for m in range(2):
                    a = actp.tile([128, NG], f32, tag=f"a0{m}")
                    nc.scalar.activation(a[:, 0:256], g0[m][:, 0:256],
                                         mybir.ActivationFunctionType.Sigmoid)
                    nc.scalar.activation(a[:, 256:384], g0[m][:, 256:384],
                                         mybir.ActivationFunctionType.Tanh)
                    nc.scalar.activation(a[:, 384:512], g0[m][:, 384:512],
                                         mybir.ActivationFunctionType.Sigmoid)
                    ig = tmp.tile([128, SL], f32, tag=f"ig0{m}")
                    nc.vector.tensor_mul(ig[:], a[:, 0:128], a[:, 256:384])
                    fc_ = tmp.tile([128, SL], f32, tag=f"fc0{m}")
                    nc.vector.tensor_mul(fc_[:], a[:, 128:256], c_cur[0][m][:])
                    cn = cst.tile([128, SL], f32, tag=f"c0{m}")
                    nc.vector.tensor_add(cn[:], ig[:], fc_[:])
                    c_cur[0][m] = cn
                    tc_ = tmp.tile([128, SL], f32, tag=f"tc0{m}")
                    nc.scalar.activation(tc_[:], cn[:],
                                         mybir.ActivationFunctionType.Tanh)
                    hm = tmp.tile([128, SL], f32, tag=f"h0{m}")
                    nc.vector.tensor_mul(hm[:], a[:, 384:512], tc_[:])
                    # transpose [b, h] -> [h, b] on PE; psum->sbuf copy rounds to f32r
                    trp = pst.tile([128, 128], f32, tag="tr")
                    nc.tensor.transpose(trp[:], hm[:], eye_sb[:])
                    nc.vector.tensor_copy(hto0[:, m * 128:(m + 1) * 128], trp[:])

                nc.gpsimd.dma_start(ag_in0[:], hto0[:])

                # ---- AllGather h0T ----
                ag_out0 = agd.tile([128 * N_CORES, B], f32r, tag="ago0")
                nc.gpsimd.collective_compute(
                    "AllGather", mybir.AluOpType.bypass,
                    replica_groups=[list(range(N_CORES))],
                    ins=[ag_in0.opt()], outs=[ag_out0.opt()],
                )
                h0t_new = state.tile([128, KT * B], f32r, tag="h0t")
                nc.sync.dma_start(
                    h0t_new[:].rearrange("p (k b) -> p k b", k=KT),
                    ag_out0[:].rearrange("(k p) b -> p k b", p=128),
                )

                # ---- layer 1 gates ----
                g1 = [psg.tile([128, NG], f32, tag=f"g1{m}", name=f"g1{m}") for m in range(2)]
                for m in range(2):
                    # h1 part first (no dependency on this step's gather)
                    for k in range(KT):
                        nc.tensor.matmul(
                            g1[m][:],
                            h1t_cur[:, k * B + m * 128: k * B + m * 128 + 128],
                            w1_r[:, (8 + k) * NG:(9 + k) * NG],
                            start=(k == 0), stop=False,
                        )
                # FC for previous step fills the PE queue while AG(h0T) lands
                if t > 0:
                    _emit_fc(nc, psf, tmp, wfc_r, bfc_sb, h1t_cur, preds_out, t - 1)
                for m in range(2):
                    for k in range(KT):
                        nc.tensor.matmul(
                            g1[m][:],
                            h0t_new[:, k * B + m * 128: k * B + m * 128 + 128],
                            w1_r[:, k * NG:(k + 1) * NG],
                            start=False, stop=False,
                        )
                    nc.tensor.matmul(
                        g1[m][:], ones_r[:], brow1, start=False, stop=True,
                    )

                # ---- layer 1 activations + cell + h1 ----
                ag_in1 = agd.tile([128, B], f32r, tag="agi1")
                hto1 = tmp.tile([128, B], f32r, tag="hto1")
                for m in range(2):
                    a = actp.tile([128, NG], f32, tag=f"a1{m}")
                    nc.scalar.activation(a[:, 0:256], g1[m][:, 0:256],
                                         mybir.ActivationFunctionType.Sigmoid)
                    nc.scalar.activation(a[:, 256:384], g1[m][:, 256:384],
                                         mybir.ActivationFunctionType.Tanh)
                    nc.scalar.activation(a[:, 384:512], g1[m][:, 384:512],
                                         mybir.ActivationFunctionType.Sigmoid)
                    ig = tmp.tile([128, SL], f32, tag=f"ig1{m}")
                    nc.vector.tensor_mul(ig[:], a[:, 0:128], a[:, 256:384])
                    fc_ = tmp.tile([128, SL], f32, tag=f"fc1{m}")
                    nc.vector.tensor_mul(fc_[:], a[:, 128:256], c_cur[1][m][:])
                    cn = cst.tile([128, SL], f32, tag=f"c1{m}")
                    nc.vector.tensor_add(cn[:], ig[:], fc_[:])
                    c_cur[1][m] = cn
                    tc_ = tmp.tile([128, SL], f32, tag=f"tc1{m}")
                    nc.scalar.activation(tc_[:], cn[:],
                                         mybir.ActivationFunctionType.Tanh)
                    hm = tmp.tile([128, SL], f32, tag=f"h1{m}")
                    nc.vector.tensor_mul(hm[:], a[:, 384:512], tc_[:])
                    trp = pst.tile([128, 128], f32, tag="tr")
                    nc.tensor.transpose(trp[:], hm[:], eye_sb[:])
                    nc.vector.tensor_copy(hto1[:, m * 128:(m + 1) * 128], trp[:])

                nc.gpsimd.dma_start(ag_in1[:], hto1[:])

                # ---- AllGather h1T ----
                ag_out1 = agd.tile([128 * N_CORES, B], f32r, tag="ago1")
                nc.gpsimd.collective_compute(
                    "AllGather", mybir.AluOpType.bypass,
                    replica_groups=[list(range(N_CORES))],
                    ins=[ag_in1.opt()], outs=[ag_out1.opt()],
                )
                h1t_new = state.tile([128, KT * B], f32r, tag="h1t")
                nc.sync.dma_start(
                    h1t_new[:].rearrange("p (k b) -> p k b", k=KT),
                    ag_out1[:].rearrange("(k p) b -> p k b", p=128),
                )

                h0t_cur, h1t_cur = h0t_new, h1t_new

            # final step's pred
            _emit_fc(nc, psf, tmp, wfc_r, bfc_sb, h1t_cur, preds_out, T - 1)

    nc.compile()
    return nc


def _emit_fc(nc, psf, tmp, wfc_r, bfc_sb, h1t, preds_out, t):
    """predT slice [FS, B] = W_fc[8r:8r+8] @ h1_t^T + b_fc[8r:8r+8]."""
    f32 = dt.float32
    pfc = psf.tile([FS, B], f32, tag="pfc")
    for k in range(KT):
        nc.tensor.matmul(
            pfc[:], wfc_r[:, k * FS:(k + 1) * FS],
            h1t[:, k * B:(k + 1) * B],
            start=(k == 0), stop=(k == KT - 1),
        )
    po = tmp.tile([FS, B], f32, tag="po")
    nc.scalar.activation(po[:], pfc[:], mybir.ActivationFunctionType.Identity,
                         bias=bfc_sb[:])
    nc.sync.dma_start(preds_out[t, :, :], po[:])


def _prep_inputs(decoder_input, hidden, cell, W_ih0, W_hh0, b_ih0, b_hh0,
                 W_ih1, W_hh1, b_ih1, b_hh1, W_fc, b_fc):
    """Host-side sharding: per-core input maps."""
    f32 = np.float32
    b0 = (b_ih0 + b_hh0).astype(f32)
    b1 = (b_ih1 + b_hh1).astype(f32)
    x0 = np.ascontiguousarray(decoder_input[:, 0, :].astype(f32))  # [B, F]
    in_maps = []
    for r in range(N_CORES):
        idx = np.concatenate(
            [np.arange(g * H + r * SL, g * H + r * SL + SL) for g in range(4)]
        )
        Wih0_sl = W_ih0[idx]            # [512, 64]
        Whh0_sl = W_hh0[idx]            # [512, 1024]
        Wih1_sl = W_ih1[idx]
        Whh1_sl = W_hh1[idx]
        W0eff = (Wih0_sl.astype(np.float64) @ W_fc.astype(np.float64)).astype(f32)

        def ktiles(WT):  # W_sl [512, K] -> [K/128, 128, 512] tiles of W_sl.T
            WT = np.ascontiguousarray(WT.T.astype(f32))  # [K, 512]
            return WT.reshape(-1, 128, NG)

        w0 = np.concatenate([ktiles(Whh0_sl), ktiles(W0eff)], axis=0)  # [16,128,512]
        w1 = np.concatenate([ktiles(Wih1_sl), ktiles(Whh1_sl)], axis=0)
        brow0_t0 = b0[idx]
        brow0 = (b0[idx].astype(np.float64)
                 + Wih0_sl.astype(np.float64) @ b_fc.astype(np.float64)).astype(f32)
        brow1 = b1[idx]
        brows = np.stack([brow0_t0, brow0, brow1])  # [3, 512]

        wfc = np.ascontiguousarray(W_fc[r * FS:(r + 1) * FS, :].T.astype(f32))  # [H, FS]
        bfc = np.ascontiguousarray(b_fc[r * FS:(r + 1) * FS].astype(f32)).reshape(FS, 1)

        in_maps.append({
            "w0_in": np.ascontiguousarray(w0),
            "w1_in": np.ascontiguousarray(w1),
            "wih0_in": np.ascontiguousarray(Wih0_sl.T.astype(f32)),
            "brows_in": np.ascontiguousarray(brows),
            "wfc_in": wfc,
            "bfc_in": bfc,
            "x0t_in": np.ascontiguousarray(x0.T),
            "h0t_in": np.ascontiguousarray(hidden[0].astype(f32).T),   # [H, B]
            "h1t_in": np.ascontiguousarray(hidden[1].astype(f32).T),
            "c0_in": np.ascontiguousarray(cell[0][:, r * SL:(r + 1) * SL].astype(f32)),
            "c1_in": np.ascontiguousarray(cell[1][:, r * SL:(r + 1) * SL].astype(f32)),
            "ones_in": np.ones((1, 128), f32),
            "eye_in": np.eye(128, dtype=f32),
        })
    return in_maps


def kernel(decoder_input, hidden, cell, W_ih0, W_hh0, b_ih0, b_hh0,
           W_ih1, W_hh1, b_ih1, b_hh1, W_fc, b_fc, output_window,
           _trace=False):
    T = int(output_window)
    in_maps = _prep_inputs(
        np.asarray(decoder_input), np.asarray(hidden), np.asarray(cell),
        np.asarray(W_ih0), np.asarray(W_hh0), np.asarray(b_ih0),
        np.asarray(b_hh0), np.asarray(W_ih1), np.asarray(W_hh1),
        np.asarray(b_ih1), np.asarray(b_hh1), np.asarray(W_fc),
        np.asarray(b_fc))

    if T not in _BUILD_CACHE:
        _BUILD_CACHE[T] = _build(T)
    nc = _BUILD_CACHE[T]

    res = run_bass_kernel_spmd(nc, in_maps, list(range(N_CORES)),
                               trace=_trace)
    # preds per core: [T, FS, B]; out[b, t, r*FS + j] = preds_r[t, j, b]
    preds = np.stack([res.results[r]["preds"] for r in range(N_CORES)])
    out = np.transpose(preds, (3, 1, 0, 2)).reshape(B, T, F)
    if _trace:
        kernel._last_results = res
    return np.ascontiguousarray(out)


# revision 14
# speedup vs baseline: 19.0973x; 19.0973x over previous
"""Trainium2 Bass kernel for nn_DecoderLSTM (2-layer LSTM decoder, B=256, F=64,
H=1024, T=96) on 8 NeuronCores.

Strategy: tensor-parallel over the 4H gate dimension (the three big weight
matrices total 50MB fp32 and do not fit in one core's 24MB SBUF). Each core
holds a 512-row slice of every gate matrix (rows [g*H + 128r, g*H + 128r + 128)
for gate g in i,f,g,o), computes its gate slice for the FULL batch B=256
(full 128-wide PE utilization), updates its h/c slice, transposes it on the PE,
and AllGathers the transposed hidden state (h^T, [1024, 256]) once per layer
per step so every core has the full contraction operand for the next matmuls.

The per-step output projection x_t = pred_{t-1} = h1 @ W_fc.T + b_fc is folded
into layer 0's weights (W0eff = W_ih0_slice @ W_fc), removing the FC from the
recurrent critical path; pred_t itself is computed per-step as an 8-row slice
per core (predT = W_fc[8r:8r+8] @ h1^T) so the 8 cores jointly produce the
full [B, T, F] output with no extra gather.

Matmuls run in float32r (TF32-like fast fp32 mode, 4x the fp32 rate, ~1e-4
relative error); the cell state c stays fp32.
"""
import sys

sys.path.insert(0, "/opt/trn_rl_repo")

import numpy as np
import concourse.bass as bass
import concourse.bacc as bacc
import concourse.mybir as mybir
import concourse.tile as tile
from concourse.bass_utils import run_bass_kernel_spmd

dt = mybir.dt

N_CORES = 8
B = 256
F = 64
H = 1024
NG = 512          # per-core gate rows (4 gates x 128)
SL = 128          # per-core within-gate slice (H / N_CORES)
KT = H // 128     # 8 K-tiles per state
FS = F // N_CORES  # 8 pred rows per core

_BUILD_CACHE = {}


def _build(T: int, no_cc: bool = False, shared_ag: bool = False):
    """Build + compile the SPMD program for T timesteps."""
    nc = bacc.Bacc(None, num_devices=N_CORES)
    f32, f32r = dt.float32, dt.float32r

    # ---- per-core external inputs ----
    w0_in = nc.dram_tensor("w0_in", [16, 128, NG], f32, kind="ExternalInput")
    w1_in = nc.dram_tensor("w1_in", [16, 128, NG], f32, kind="ExternalInput")
    wih0_in = nc.dram_tensor("wih0_in", [F, NG], f32, kind="ExternalInput")
    brows_in = nc.dram_tensor("brows_in", [3, NG], f32, kind="ExternalInput")  # b0_t0, b0eff, b1
    wfc_in = nc.dram_tensor("wfc_in", [H, FS], f32, kind="ExternalInput")
    bfc_in = nc.dram_tensor("bfc_in", [FS, 1], f32, kind="ExternalInput")
    x0t_in = nc.dram_tensor("x0t_in", [F, B], f32, kind="ExternalInput")
    h0t_in = nc.dram_tensor("h0t_in", [H, B], f32, kind="ExternalInput")
    h1t_in = nc.dram_tensor("h1t_in", [H, B], f32, kind="ExternalInput")
    c0_in = nc.dram_tensor("c0_in", [B, SL], f32, kind="ExternalInput")
    c1_in = nc.dram_tensor("c1_in", [B, SL], f32, kind="ExternalInput")
    ones_in = nc.dram_tensor("ones_in", [1, 128], f32, kind="ExternalInput")
    eye_in = nc.dram_tensor("eye_in", [128, 128], f32, kind="ExternalInput")

    preds_out = nc.dram_tensor("preds", [T, FS, B], f32, kind="ExternalOutput")

    with tile.TileContext(nc) as tc:
        with (
            tc.tile_pool(name="wpool", bufs=1) as wpool,      # persistent weights
            tc.tile_pool(name="state", bufs=2) as state,      # hT double buffers
            tc.tile_pool(name="cst", bufs=2) as cst,          # c state tiles
            tc.tile_pool(name="act", bufs=3) as actp,         # activation tiles
            tc.tile_pool(name="tmp", bufs=3) as tmp,          # small temporaries
            tc.tile_pool(name="psg", bufs=1, space="PSUM") as psg,
            tc.tile_pool(name="pst", bufs=2, space="PSUM") as pst,
            tc.tile_pool(name="psf", bufs=2, space="PSUM") as psf,
            tc.tile_pool(name="agd", bufs=4, space="DRAM") as agd,
        ):
            # ================= init: load + round weights to f32r =============
            w0_r = wpool.tile([128, 16 * NG], f32r)
            w1_r = wpool.tile([128, 16 * NG], f32r)
            with tc.tile_pool(name="stage", bufs=2) as stage:
                for src, dst in ((w0_in, w0_r), (w1_in, w1_r)):
                    for k in range(16):
                        st = stage.tile([128, NG], f32, tag="wst")
                        nc.sync.dma_start(st[:], src[k, :, :])
                        nc.vector.tensor_copy(dst[:, k * NG:(k + 1) * NG], st[:])

                wih0_r = wpool.tile([F, NG], f32r)
                st = stage.tile([F, NG], f32, tag="mst")
                nc.sync.dma_start(st[:], wih0_in[:])
                nc.vector.tensor_copy(wih0_r[:], st[:])

                brow_tiles = []
                for bi in range(3):
                    br = wpool.tile([1, NG], f32r, tag=f"brow{bi}", name=f"brow{bi}")
                    st = stage.tile([1, NG], f32, tag="mst")
                    nc.sync.dma_start(st[:], brows_in[bi:bi + 1, :])
                    nc.vector.tensor_copy(br[:], st[:])
                    brow_tiles.append(br)

                wfc_r = wpool.tile([128, KT * FS], f32r)
                st = stage.tile([128, KT * FS], f32, tag="mst")
                nc.sync.dma_start(
                    st[:].rearrange("p (k f) -> p k f", k=KT),
                    wfc_in[:].rearrange("(k p) f -> p k f", p=128),
                )
                nc.vector.tensor_copy(wfc_r[:], st[:])

                bfc_sb = wpool.tile([FS, 1], f32)
                nc.sync.dma_start(bfc_sb[:], bfc_in[:])

                x0t_r = wpool.tile([F, B], f32r)
                st = stage.tile([F, B], f32, tag="mst")
                nc.sync.dma_start(st[:], x0t_in[:])
                nc.vector.tensor_copy(x0t_r[:], st[:])

                ones_r = wpool.tile([1, 128], f32r)
                st = stage.tile([1, 128], f32, tag="mst")
                nc.sync.dma_start(st[:], ones_in[:])
                nc.vector.tensor_copy(ones_r[:], st[:])

                eye_sb = wpool.tile([128, 128], f32)
                nc.sync.dma_start(eye_sb[:], eye_in[:])

                # initial transposed states (full, gathered layout) and c slices
                h0t_cur = state.tile([128, KT * B], f32r, tag="h0t")
                h1t_cur = state.tile([128, KT * B], f32r, tag="h1t")
                for src, dst in ((h0t_in, h0t_cur), (h1t_in, h1t_cur)):
                    st = stage.tile([128, KT * B], f32, tag="hst")
                    nc.sync.dma_start(
                        st[:].rearrange("p (k b) -> p k b", k=KT),
                        src[:].rearrange("(k p) b -> p k b", p=128),
                    )
                    nc.vector.tensor_copy(dst[:], st[:])

                c_cur = [[None, None], [None, None]]
                for li, src in ((0, c0_in), (1, c1_in)):
                    for m in range(2):
                        ct = cst.tile([128, SL], f32, tag=f"c{li}{m}")
                        nc.sync.dma_start(ct[:], src[m * 128:(m + 1) * 128, :])
                        c_cur[li][m] = ct

            # bias rows ([1, NG] f32r)
            brow0_t0 = brow_tiles[0][:]
            brow0 = brow_tiles[1][:]
            brow1 = brow_tiles[2][:]

            # ================= recurrent steps ================================
            for t in range(T):
                # ---- layer 0 gates: [2][128, 512] psum ----
                g0 = [psg.tile([128, NG], f32, tag=f"g0{m}", name=f"g0{m}") for m in range(2)]
                for m in range(2):
                    # h0 part (k-tiles 0..7) — prev-step h0T, gathered early
                    for k in range(KT):
                        nc.tensor.matmul(
                            g0[m][:],
                            h0t_cur[:, k * B + m * 128: k * B + m * 128 + 128],
                            w0_r[:, k * NG:(k + 1) * NG],
                            start=(k == 0), stop=False,
                        )
                    if t == 0:
                        # x0 path: K=64 stationary + its own bias row
                        nc.tensor.matmul(
                            g0[m][:], x0t_r[:, m * 128: m * 128 + 128],
                            wih0_r[:], start=False, stop=False,
                        )
                        nc.tensor.matmul(
                            g0[m][:], ones_r[:], brow0_t0,
                            start=False, stop=True,
                        )
                    else:
                        # h1 part via fused W0eff (k-tiles 8..15) + bias row
                        for k in range(KT):
                            nc.tensor.matmul(
                                g0[m][:],
                                h1t_cur[:, k * B + m * 128: k * B + m * 128 + 128],
                                w0_r[:, (8 + k) * NG:(9 + k) * NG],
                                start=False, stop=False,
                            )
                        nc.tensor.matmul(
                            g0[m][:], ones_r[:], brow0,
                            start=False, stop=True,
                        )

                # ---- layer 0 activations + cell update + h0 ----
                ag_in0 = agd.tile([128, B], f32r, tag="agi0")
                hto0 = tmp.tile([128, B], f32r, tag="hto0")
                for m in range(2):
                    a = actp.tile([128, NG], f32, tag=f"a0{m}")
                    nc.scalar.activation(a[:, 0:256], g0[m][:, 0:256],
                                         mybir.ActivationFunctionType.Sigmoid)
                    nc.scalar.activation(a[:, 256:384], g0[m][:, 256:384],
                                         mybir.ActivationFunctionType.Tanh)
                    nc.scalar.activation(a[:, 384:512], g0[m][:, 384:512],
                                         mybir.ActivationFunctionType.Sigmoid)
                    ig = tmp.tile([128, SL], f32, tag=f"ig0{m}")
                    nc.vector.tensor_mul(ig[:], a[:, 0:128], a[:, 256:384])
                    fc_ = tmp.tile([128, SL], f32, tag=f"fc0{m}")
                    nc.vector.tensor_mul(fc_[:], a[:, 128:256], c_cur[0][m][:])
                    cn = cst.tile([128, SL], f32, tag=f"c0{m}")
                    nc.vector.tensor_add(cn[:], ig[:], fc_[:])
                    c_cur[0][m] = cn
                    tc_ = tmp.tile([128, SL], f32, tag=f"tc0{m}")
                    nc.scalar.activation(tc_[:], cn[:],
                                         mybir.ActivationFunctionType.Tanh)
                    hm = tmp.tile([128, SL], f32, tag=f"h0{m}")
                    nc.vector.tensor_mul(hm[:], a[:, 384:512], tc_[:])
                    # transpose [b, h] -> [h, b] on PE; psum->sbuf copy rounds to f32r
                    trp = pst.tile([128, 128], f32, tag="tr")
                    nc.tensor.transpose(trp[:], hm[:], eye_sb[:])
                    nc.vector.tensor_copy(hto0[:, m * 128:(m + 1) * 128], trp[:])

                nc.gpsimd.dma_start(ag_in0[:], hto0[:])

                # ---- AllGather h0T ----
                ag_out0 = agd.tile([128 * N_CORES, B], f32r, tag="ago0",
                                      addr_space="Shared" if shared_ag else "Local")
                if no_cc:
                    for _rr in range(N_CORES):
                        nc.gpsimd.dma_start(
                            ag_out0[_rr * 128:(_rr + 1) * 128, :], ag_in0[:])
                else:
                    nc.gpsimd.collective_compute(
                        "AllGather", mybir.AluOpType.bypass,
                        replica_groups=[list(range(N_CORES))],
                        ins=[ag_in0.opt()], outs=[ag_out0.opt()],
                    )
                h0t_new = state.tile([128, KT * B], f32r, tag="h0t")
                nc.sync.dma_start(
                    h0t_new[:].rearrange("p (k b) -> p k b", k=KT),
                    ag_out0[:].rearrange("(k p) b -> p k b", p=128),
                )

                # ---- layer 1 gates ----
                g1 = [psg.tile([128, NG], f32, tag=f"g1{m}", name=f"g1{m}") for m in range(2)]
                for m in range(2):
                    # h1 part first (no dependency on this step's gather)
                    for k in range(KT):
                        nc.tensor.matmul(
                            g1[m][:],
                            h1t_cur[:, k * B + m * 128: k * B + m * 128 + 128],
                            w1_r[:, (8 + k) * NG:(9 + k) * NG],
                            start=(k == 0), stop=False,
                        )
                # FC for previous step fills the PE queue while AG(h0T) lands
                if t > 0:
                    _emit_fc(nc, psf, tmp, wfc_r, bfc_sb, h1t_cur, preds_out, t - 1)
                for m in range(2):
                    for k in range(KT):
                        nc.tensor.matmul(
                            g1[m][:],
                            h0t_new[:, k * B + m * 128: k * B + m * 128 + 128],
                            w1_r[:, k * NG:(k + 1) * NG],
                            start=False, stop=False,
                        )
                    nc.tensor.matmul(
                        g1[m][:], ones_r[:], brow1, start=False, stop=True,
                    )

                # ---- layer 1 activations + cell + h1 ----
                ag_in1 = agd.tile([128, B], f32r, tag="agi1")
                hto1 = tmp.tile([128, B], f32r, tag="hto1")
                for m in range(2):
                    a = actp.tile([128, NG], f32, tag=f"a1{m}")
                    nc.scalar.activation(a[:, 0:256], g1[m][:, 0:256],
                                         mybir.ActivationFunctionType.Sigmoid)
                    nc.scalar.activation(a[:, 256:384], g1[m][:, 256:384],
                                         mybir.ActivationFunctionType.Tanh)
                    nc.scalar.activation(a[:, 384:512], g1[m][:, 384:512],
                                         mybir.ActivationFunctionType.Sigmoid)
                    ig = tmp.tile([128, SL], f32, tag=f"ig1{m}")
                    nc.vector.tensor_mul(ig[:], a[:, 0:128], a[:, 256:384])
                    fc_ = tmp.tile([128, SL], f32, tag=f"fc1{m}")
                    nc.vector.tensor_mul(fc_[:], a[:, 128:256], c_cur[1][m][:])
                    cn = cst.tile([128, SL], f32, tag=f"c1{m}")
                    nc.vector.tensor_add(cn[:], ig[:], fc_[:])
                    c_cur[1][m] = cn
                    tc_ = tmp.tile([128, SL], f32, tag=f"tc1{m}")
                    nc.scalar.activation(tc_[:], cn[:],
                                         mybir.ActivationFunctionType.Tanh)
                    hm = tmp.tile([128, SL], f32, tag=f"h1{m}")
                    nc.vector.tensor_mul(hm[:], a[:, 384:512], tc_[:])
                    trp = pst.tile([128, 128], f32, tag="tr")
                    nc.tensor.transpose(trp[:], hm[:], eye_sb[:])
                    nc.vector.tensor_copy(hto1[:, m * 128:(m + 1) * 128], trp[:])

                nc.gpsimd.dma_start(ag_in1[:], hto1[:])

                # ---- AllGather h1T ----
                ag_out1 = agd.tile([128 * N_CORES, B], f32r, tag="ago1",
                                      addr_space="Shared" if shared_ag else "Local")
                if no_cc:
                    for _rr in range(N_CORES):
                        nc.gpsimd.dma_start(
                            ag_out1[_rr * 128:(_rr + 1) * 128, :], ag_in1[:])
                else:
                    nc.gpsimd.collective_compute(
                        "AllGather", mybir.AluOpType.bypass,
                        replica_groups=[list(range(N_CORES))],
                        ins=[ag_in1.opt()], outs=[ag_out1.opt()],
                    )
                h1t_new = state.tile([128, KT * B], f32r, tag="h1t")
                nc.sync.dma_start(
                    h1t_new[:].rearrange("p (k b) -> p k b", k=KT),
                    ag_out1[:].rearrange("(k p) b -> p k b", p=128),
                )

                h0t_cur, h1t_cur = h0t_new, h1t_new

            # final step's pred
            _emit_fc(nc, psf, tmp, wfc_r, bfc_sb, h1t_cur, preds_out, T - 1)

    nc.compile()
    return nc


def _emit_fc(nc, psf, tmp, wfc_r, bfc_sb, h1t, preds_out, t):
    """predT slice [FS, B] = W_fc[8r:8r+8] @ h1_t^T + b_fc[8r:8r+8]."""
    f32 = dt.float32
    pfc = psf.tile([FS, B], f32, tag="pfc")
    for k in range(KT):
        nc.tensor.matmul(
            pfc[:], wfc_r[:, k * FS:(k + 1) * FS],
            h1t[:, k * B:(k + 1) * B],
            start=(k == 0), stop=(k == KT - 1),
        )
    po = tmp.tile([FS, B], f32, tag="po")
    nc.scalar.activation(po[:], pfc[:], mybir.ActivationFunctionType.Identity,
                         bias=bfc_sb[:])
    nc.sync.dma_start(preds_out[t, :, :], po[:])


def _prep_inputs(decoder_input, hidden, cell, W_ih0, W_hh0, b_ih0, b_hh0,
                 W_ih1, W_hh1, b_ih1, b_hh1, W_fc, b_fc):
    """Host-side sharding: per-core input maps."""
    f32 = np.float32
    b0 = (b_ih0 + b_hh0).astype(f32)
    b1 = (b_ih1 + b_hh1).astype(f32)
    x0 = np.ascontiguousarray(decoder_input[:, 0, :].astype(f32))  # [B, F]
    in_maps = []
    for r in range(N_CORES):
        idx = np.concatenate(
            [np.arange(g * H + r * SL, g * H + r * SL + SL) for g in range(4)]
        )
        Wih0_sl = W_ih0[idx]            # [512, 64]
        Whh0_sl = W_hh0[idx]            # [512, 1024]
        Wih1_sl = W_ih1[idx]
        Whh1_sl = W_hh1[idx]
        W0eff = (Wih0_sl.astype(np.float64) @ W_fc.astype(np.float64)).astype(f32)

        def ktiles(WT):  # W_sl [512, K] -> [K/128, 128, 512] tiles of W_sl.T
            WT = np.ascontiguousarray(WT.T.astype(f32))  # [K, 512]
            return WT.reshape(-1, 128, NG)

        w0 = np.concatenate([ktiles(Whh0_sl), ktiles(W0eff)], axis=0)  # [16,128,512]
        w1 = np.concatenate([ktiles(Wih1_sl), ktiles(Whh1_sl)], axis=0)
        brow0_t0 = b0[idx]
        brow0 = (b0[idx].astype(np.float64)
                 + Wih0_sl.astype(np.float64) @ b_fc.astype(np.float64)).astype(f32)
        brow1 = b1[idx]
        brows = np.stack([brow0_t0, brow0, brow1])  # [3, 512]

        wfc = np.ascontiguousarray(W_fc[r * FS:(r + 1) * FS, :].T.astype(f32))  # [H, FS]
        bfc = np.ascontiguousarray(b_fc[r * FS:(r + 1) * FS].astype(f32)).reshape(FS, 1)

        in_maps.append({
            "w0_in": np.ascontiguousarray(w0),
            "w1_in": np.ascontiguousarray(w1),
            "wih0_in": np.ascontiguousarray(Wih0_sl.T.astype(f32)),
            "brows_in": np.ascontiguousarray(brows),
            "wfc_in": wfc,
            "bfc_in": bfc,
            "x0t_in": np.ascontiguousarray(x0.T),
            "h0t_in": np.ascontiguousarray(hidden[0].astype(f32).T),   # [H, B]
            "h1t_in": np.ascontiguousarray(hidden[1].astype(f32).T),
            "c0_in": np.ascontiguousarray(cell[0][:, r * SL:(r + 1) * SL].astype(f32)),
            "c1_in": np.ascontiguousarray(cell[1][:, r * SL:(r + 1) * SL].astype(f32)),
            "ones_in": np.ones((1, 128), f32),
            "eye_in": np.eye(128, dtype=f32),
        })
    return in_maps


def kernel(decoder_input, hidden, cell, W_ih0, W_hh0, b_ih0, b_hh0,
           W_ih1, W_hh1, b_ih1, b_hh1, W_fc, b_fc, output_window,
           _trace=False):
    T = int(output_window)
    in_maps = _prep_inputs(
        np.asarray(decoder_input), np.asarray(hidden), np.asarray(cell),
        np.asarray(W_ih0), np.asarray(W_hh0), np.asarray(b_ih0),
        np.asarray(b_hh0), np.asarray(W_ih1), np.asarray(W_hh1),
        np.asarray(b_ih1), np.asarray(b_hh1), np.asarray(W_fc),
        np.asarray(b_fc))

    if T not in _BUILD_CACHE:
        _BUILD_CACHE[T] = _build(T)
    nc = _BUILD_CACHE[T]

    res = run_bass_kernel_spmd(nc, in_maps, list(range(N_CORES)),
                               trace=_trace)
    # preds per core: [T, FS, B]; out[b, t, r*FS + j] = preds_r[t, j, b]
    preds = np.stack([res.results[r]["preds"] for r in range(N_CORES)])
    out = np.transpose(preds, (3, 1, 0, 2)).reshape(B, T, F)
    if _trace:
        kernel._last_results = res
    return np.ascontiguousarray(out)


# revision 15
# speedup vs baseline: 21.1443x; 1.1072x over previous
"""Trainium2 Bass kernel for nn_DecoderLSTM (2-layer LSTM decoder, B=256, F=64,
H=1024, T=96) on 8 NeuronCores.

Strategy: tensor-parallel over the 4H gate dimension (the three big weight
matrices total 50MB fp32 and do not fit in one core's 24MB SBUF). Each core
holds a 512-row slice of every gate matrix (rows [g*H + 128r, g*H + 128r + 128)
for gate g in i,f,g,o), computes its gate slice for the FULL batch B=256
(full 128-wide PE utilization), updates its h/c slice, transposes it on the PE,
and AllGathers the transposed hidden state (h^T, [1024, 256]) once per layer
per step so every core has the full contraction operand for the next matmuls.

Every core computes the full transposed projection predT = W_fc @ h1^T + b_fc
each step (it is both the kernel output and the next step's layer-0 input x);
core 0's copy is returned.

Matmuls run in float32r (TF32-like fast fp32 mode, ~1.4e-4 relative error);
the cell state c stays fp32.
"""
import sys

sys.path.insert(0, "/opt/trn_rl_repo")

import numpy as np
import concourse.bass as bass
import concourse.bacc as bacc
import concourse.mybir as mybir
import concourse.tile as tile
from concourse.bass_utils import run_bass_kernel_spmd

dt = mybir.dt
AF = mybir.ActivationFunctionType

N_CORES = 8
B = 256
F = 64
H = 1024
NG = 512          # per-core gate rows (4 gates x 128)
SL = 128          # per-core within-gate slice (H / N_CORES)
KT = H // 128     # 8 K-tiles per state

_BUILD_CACHE = {}


def _build(T: int, no_cc: bool = False, shared_ag: bool = True):
    """Build + compile the SPMD program for T timesteps."""
    nc = bacc.Bacc(None, num_devices=N_CORES)
    f32, f32r = dt.float32, dt.float32r

    # ---- per-core external inputs ----
    w0_in = nc.dram_tensor("w0_in", [KT, 128, NG], f32, kind="ExternalInput")
    w1_in = nc.dram_tensor("w1_in", [2 * KT, 128, NG], f32, kind="ExternalInput")
    wih0_in = nc.dram_tensor("wih0_in", [F, NG], f32, kind="ExternalInput")
    brows_in = nc.dram_tensor("brows_in", [2, NG], f32, kind="ExternalInput")  # b0, b1
    wfc_in = nc.dram_tensor("wfc_in", [H, F], f32, kind="ExternalInput")
    bfc_in = nc.dram_tensor("bfc_in", [F, 1], f32, kind="ExternalInput")
    x0t_in = nc.dram_tensor("x0t_in", [F, B], f32, kind="ExternalInput")
    h0t_in = nc.dram_tensor("h0t_in", [H, B], f32, kind="ExternalInput")
    h1t_in = nc.dram_tensor("h1t_in", [H, B], f32, kind="ExternalInput")
    c0_in = nc.dram_tensor("c0_in", [B, SL], f32, kind="ExternalInput")
    c1_in = nc.dram_tensor("c1_in", [B, SL], f32, kind="ExternalInput")
    ones_in = nc.dram_tensor("ones_in", [1, 128], f32, kind="ExternalInput")
    eye_in = nc.dram_tensor("eye_in", [128, 128], f32, kind="ExternalInput")

    preds_out = nc.dram_tensor("preds", [T, F, B], f32, kind="ExternalOutput")

    ag_space = "Shared" if shared_ag else "Local"

    with tile.TileContext(nc) as tc:
        with (
            tc.tile_pool(name="wpool", bufs=1) as wpool,      # persistent weights
            tc.tile_pool(name="state", bufs=2) as state,      # hT double buffers
            tc.tile_pool(name="cst", bufs=2) as cst,          # c state tiles
            tc.tile_pool(name="act", bufs=3) as actp,         # activation tiles
            tc.tile_pool(name="tmp", bufs=3) as tmp,          # small temporaries
            tc.tile_pool(name="psg", bufs=1, space="PSUM") as psg,
            tc.tile_pool(name="pst", bufs=2, space="PSUM") as pst,
            tc.tile_pool(name="psf", bufs=2, space="PSUM") as psf,
            tc.tile_pool(name="agd", bufs=4, space="DRAM") as agd,
        ):
            # ================= init: load + round weights to f32r =============
            w0_r = wpool.tile([128, KT * NG], f32r)        # layer0 h0-part
            w1_r = wpool.tile([128, 2 * KT * NG], f32r)    # layer1 h0+h1 parts
            with tc.tile_pool(name="stage", bufs=2) as stage:
                for k in range(KT):
                    st = stage.tile([128, NG], f32, tag="wst")
                    nc.sync.dma_start(st[:], w0_in[k, :, :])
                    nc.vector.tensor_copy(w0_r[:, k * NG:(k + 1) * NG], st[:])
                for k in range(2 * KT):
                    st = stage.tile([128, NG], f32, tag="wst")
                    nc.sync.dma_start(st[:], w1_in[k, :, :])
                    nc.vector.tensor_copy(w1_r[:, k * NG:(k + 1) * NG], st[:])

                wih0_r = wpool.tile([F, NG], f32r)
                st = stage.tile([F, NG], f32, tag="mst")
                nc.sync.dma_start(st[:], wih0_in[:])
                nc.vector.tensor_copy(wih0_r[:], st[:])

                brow_tiles = []
                for bi in range(2):
                    br = wpool.tile([1, NG], f32r, tag=f"brow{bi}", name=f"brow{bi}")
                    st = stage.tile([1, NG], f32, tag="mst")
                    nc.sync.dma_start(st[:], brows_in[bi:bi + 1, :])
                    nc.vector.tensor_copy(br[:], st[:])
                    brow_tiles.append(br)

                # W_fc as K-tiles [128, F] (k-major packed)
                wfc_r = wpool.tile([128, KT * F], f32r)
                st = stage.tile([128, KT * F], f32, tag="mst")
                nc.sync.dma_start(
                    st[:].rearrange("p (k f) -> p k f", k=KT),
                    wfc_in[:].rearrange("(k p) f -> p k f", p=128),
                )
                nc.vector.tensor_copy(wfc_r[:], st[:])

                bfc_sb = wpool.tile([F, 1], f32)
                nc.sync.dma_start(bfc_sb[:], bfc_in[:])

                x0t_r = wpool.tile([F, B], f32r)
                st = stage.tile([F, B], f32, tag="mst")
                nc.sync.dma_start(st[:], x0t_in[:])
                nc.vector.tensor_copy(x0t_r[:], st[:])

                ones_r = wpool.tile([1, 128], f32r)
                st = stage.tile([1, 128], f32, tag="mst")
                nc.sync.dma_start(st[:], ones_in[:])
                nc.vector.tensor_copy(ones_r[:], st[:])

                eye_sb = wpool.tile([128, 128], f32)
                nc.sync.dma_start(eye_sb[:], eye_in[:])

                # initial transposed states (full, gathered layout) and c slices
                h0t_cur = state.tile([128, KT * B], f32r, tag="h0t")
                h1t_cur = state.tile([128, KT * B], f32r, tag="h1t")
                for src, dst in ((h0t_in, h0t_cur), (h1t_in, h1t_cur)):
                    st = stage.tile([128, KT * B], f32, tag="hst")
                    nc.sync.dma_start(
                        st[:].rearrange("p (k b) -> p k b", k=KT),
                        src[:].rearrange("(k p) b -> p k b", p=128),
                    )
                    nc.vector.tensor_copy(dst[:], st[:])

                c_cur = [[None, None], [None, None]]
                for li, src in ((0, c0_in), (1, c1_in)):
                    for m in range(2):
                        ct = cst.tile([128, SL], f32, tag=f"c{li}{m}")
                        nc.sync.dma_start(ct[:], src[m * 128:(m + 1) * 128, :])
                        c_cur[li][m] = ct

            brow0 = brow_tiles[0][:]
            brow1 = brow_tiles[1][:]

            def gate_acts(g, li, m):
                a = actp.tile([128, NG], f32, tag=f"a{li}{m}", name="a")
                nc.scalar.activation(a[:, 0:256], g[:, 0:256], AF.Sigmoid)
                nc.scalar.activation(a[:, 256:384], g[:, 256:384], AF.Tanh)
                nc.scalar.activation(a[:, 384:512], g[:, 384:512], AF.Sigmoid)
                return a

            def lstm_sub(a, li, m):
                """cell update + h for layer li, m-tile; returns h tile."""
                ig = tmp.tile([128, SL], f32, tag=f"ig{li}{m}", name="ig")
                nc.vector.tensor_mul(ig[:], a[:, 0:128], a[:, 256:384])
                fc_ = tmp.tile([128, SL], f32, tag=f"fcx{li}{m}", name="fc_")
                nc.vector.tensor_mul(fc_[:], a[:, 128:256], c_cur[li][m][:])
                cn = cst.tile([128, SL], f32, tag=f"c{li}{m}", name="cn")
                nc.vector.tensor_add(cn[:], ig[:], fc_[:])
                c_cur[li][m] = cn
                tc_ = tmp.tile([128, SL], f32, tag=f"tcx{li}{m}", name="tc_")
                nc.scalar.activation(tc_[:], cn[:], AF.Tanh)
                hm = tmp.tile([128, SL], f32, tag=f"h{li}{m}", name="hm")
                nc.vector.tensor_mul(hm[:], a[:, 384:512], tc_[:])
                return hm

            def emit_fc(h1t, tout):
                pfc = psf.tile([F, B], f32, tag="pfc", name="pfc")
                for k in range(KT):
                    nc.tensor.matmul(
                        pfc[:], wfc_r[:, k * F:(k + 1) * F],
                        h1t[:, k * B:(k + 1) * B],
                        start=(k == 0), stop=(k == KT - 1),
                    )
                po = tmp.tile([F, B], f32, tag="po", name="po")
                nc.scalar.activation(po[:], pfc[:], AF.Identity, bias=bfc_sb[:])
                nc.sync.dma_start(preds_out[tout, :, :], po[:])
                return po

            def emit_ag(ag_in, tag):
                ag_out = agd.tile([128 * N_CORES, B], f32r, tag=tag, name=tag,
                                  addr_space=ag_space)
                if no_cc:
                    for _rr in range(N_CORES):
                        nc.gpsimd.dma_start(
                            ag_out[_rr * 128:(_rr + 1) * 128, :], ag_in[:])
                else:
                    nc.gpsimd.collective_compute(
                        "AllGather", mybir.AluOpType.bypass,
                        replica_groups=[list(range(N_CORES))],
                        ins=[ag_in.opt()], outs=[ag_out.opt()],
                    )
                return ag_out

            def gather_in(ag_out, tag):
                hT = state.tile([128, KT * B], f32r, tag=tag, name=tag)
                for half in range(2):
                    ksl = slice(half * (KT // 2) * B, (half + 1) * (KT // 2) * B)
                    nc.sync.dma_start(
                        hT[:, ksl].rearrange("p (k b) -> p k b", k=KT // 2),
                        ag_out[half * 512:(half + 1) * 512, :]
                        .rearrange("(k p) b -> p k b", p=128),
                    )
                return hT

            # ================= recurrent steps ================================
            for t in range(T):
                # x_t = pred_{t-1} = W_fc @ h1T + b_fc (also the t-1 output)
                if t == 0:
                    x_r = x0t_r
                else:
                    po = emit_fc(h1t_cur, t - 1)
                    x_r = tmp.tile([F, B], f32r, tag="xr", name="x_r")
                    nc.vector.tensor_copy(x_r[:], po[:])

                # ---- layer 0 gates ----
                g0 = [psg.tile([128, NG], f32, tag=f"g0{m}", name=f"g0{m}")
                      for m in range(2)]
                for m in range(2):
                    for k in range(KT):
                        nc.tensor.matmul(
                            g0[m][:],
                            h0t_cur[:, k * B + m * 128: k * B + m * 128 + 128],
                            w0_r[:, k * NG:(k + 1) * NG],
                            start=(k == 0), stop=False,
                        )
                    nc.tensor.matmul(
                        g0[m][:], x_r[:, m * 128: m * 128 + 128],
                        wih0_r[:], start=False, stop=False,
                    )
                    nc.tensor.matmul(
                        g0[m][:], ones_r[:], brow0, start=False, stop=True,
                    )

                # ---- layer 0 activations + cell + h0; transpose + bounce ----
                ag_in0 = agd.tile([128, B], f32r, tag="agi0")
                hto0 = tmp.tile([128, B], f32r, tag="hto0")
                for m in range(2):
                    a = gate_acts(g0[m], 0, m)
                    hm = lstm_sub(a, 0, m)
                    trp = pst.tile([128, 128], f32, tag="tr", name="trp")
                    nc.tensor.transpose(trp[:], hm[:], eye_sb[:])
                    nc.vector.tensor_copy(hto0[:, m * 128:(m + 1) * 128], trp[:])
                nc.gpsimd.dma_start(ag_in0[:], hto0[:])
                ag_out0 = emit_ag(ag_in0, "ago0")
                h0t_new = gather_in(ag_out0, "h0t")

                # ---- layer 1 gates: h1 part first (indep of this step's AG) --
                g1 = [psg.tile([128, NG], f32, tag=f"g1{m}", name=f"g1{m}")
                      for m in range(2)]
                for m in range(2):
                    for k in range(KT):
                        nc.tensor.matmul(
                            g1[m][:],
                            h1t_cur[:, k * B + m * 128: k * B + m * 128 + 128],
                            w1_r[:, (KT + k) * NG:(KT + k + 1) * NG],
                            start=(k == 0), stop=False,
                        )
                for m in range(2):
                    for k in range(KT):
                        nc.tensor.matmul(
                            g1[m][:],
                            h0t_new[:, k * B + m * 128: k * B + m * 128 + 128],
                            w1_r[:, k * NG:(k + 1) * NG],
                            start=False, stop=False,
                        )
                    nc.tensor.matmul(
                        g1[m][:], ones_r[:], brow1, start=False, stop=True,
                    )

                # ---- layer 1 activations + cell + h1; transpose + bounce ----
                ag_in1 = agd.tile([128, B], f32r, tag="agi1")
                hto1 = tmp.tile([128, B], f32r, tag="hto1")
                for m in range(2):
                    a = gate_acts(g1[m], 1, m)
                    hm = lstm_sub(a, 1, m)
                    trp = pst.tile([128, 128], f32, tag="tr", name="trp")
                    nc.tensor.transpose(trp[:], hm[:], eye_sb[:])
                    nc.vector.tensor_copy(hto1[:, m * 128:(m + 1) * 128], trp[:])
                nc.gpsimd.dma_start(ag_in1[:], hto1[:])
                ag_out1 = emit_ag(ag_in1, "ago1")
                h1t_new = gather_in(ag_out1, "h1t")

                h0t_cur, h1t_cur = h0t_new, h1t_new

            emit_fc(h1t_cur, T - 1)

    nc.compile()
    return nc


def _prep_inputs(decoder_input, hidden, cell, W_ih0, W_hh0, b_ih0, b_hh0,
                 W_ih1, W_hh1, b_ih1, b_hh1, W_fc, b_fc):
    """Host-side sharding: per-core input maps."""
    f32 = np.float32
    b0 = (b_ih0 + b_hh0).astype(f32)
    b1 = (b_ih1 + b_hh1).astype(f32)
    x0 = np.ascontiguousarray(decoder_input[:, 0, :].astype(f32))  # [B, F]
    in_maps = []
    for r in range(N_CORES):
        idx = np.concatenate(
            [np.arange(g * H + r * SL, g * H + r * SL + SL) for g in range(4)]
        )

        def ktiles(W_sl):  # [512, K] -> [K/128, 128, 512] tiles of W_sl.T
            WT = np.ascontiguousarray(W_sl.T.astype(f32))
            return WT.reshape(-1, 128, NG)

        w0 = ktiles(W_hh0[idx])                                   # [8,128,512]
        w1 = np.concatenate([ktiles(W_ih1[idx]), ktiles(W_hh1[idx])], axis=0)
        brows = np.stack([b0[idx], b1[idx]])                      # [2, 512]

        in_maps.append({
            "w0_in": np.ascontiguousarray(w0),
            "w1_in": np.ascontiguousarray(w1),
            "wih0_in": np.ascontiguousarray(W_ih0[idx].T.astype(f32)),
            "brows_in": np.ascontiguousarray(brows),
            "wfc_in": np.ascontiguousarray(W_fc.T.astype(f32)),   # [H, F]
            "bfc_in": np.ascontiguousarray(b_fc.astype(f32)).reshape(F, 1),
            "x0t_in": np.ascontiguousarray(x0.T),
            "h0t_in": np.ascontiguousarray(hidden[0].astype(f32).T),   # [H, B]
            "h1t_in": np.ascontiguousarray(hidden[1].astype(f32).T),
            "c0_in": np.ascontiguousarray(cell[0][:, r * SL:(r + 1) * SL].astype(f32)),
            "c1_in": np.ascontiguousarray(cell[1][:, r * SL:(r + 1) * SL].astype(f32)),
            "ones_in": np.ones((1, 128), f32),
            "eye_in": np.eye(128, dtype=f32),
        })
    return in_maps


def kernel(decoder_input, hidden, cell, W_ih0, W_hh0, b_ih0, b_hh0,
           W_ih1, W_hh1, b_ih1, b_hh1, W_fc, b_fc, output_window,
           _trace=False):
    T = int(output_window)
    in_maps = _prep_inputs(
        np.asarray(decoder_input), np.asarray(hidden), np.asarray(cell),
        np.asarray(W_ih0), np.asarray(W_hh0), np.asarray(b_ih0),
        np.asarray(b_hh0), np.asarray(W_ih1), np.asarray(W_hh1),
        np.asarray(b_ih1), np.asarray(b_hh1), np.asarray(W_fc),
        np.asarray(b_fc))

    if T not in _BUILD_CACHE:
        _BUILD_CACHE[T] = _build(T)
    nc = _BUILD_CACHE[T]

    res = run_bass_kernel_spmd(nc, in_maps, list(range(N_CORES)),
                               trace=_trace)
    # preds from core 0: [T, F, B] -> out[b, t, f]
    preds = res.results[0]["preds"]
    out = np.ascontiguousarray(np.transpose(preds, (2, 0, 1)))
    if _trace:
        kernel._last_results = res
    return out
